# revision 6
# baseline (speedup 1.0000x reference)
"""Causal self-attention on 8 trn2 NeuronCores — fp8-DR scores + fp8-DR q/k
projections + transposed-PV emission schedule.

Sharding: tensor-parallel over heads (2 heads/core).

Key techniques (v2):
  - Score matmuls in fp8e4 DoubleRow (0.5 cyc/row): lhsT kT8 carries an
    [k_hi, k_lo] e4m3 residual pair in the DR slots, rhs is qT8 e4m3
    stride-0-broadcast into both slots.
  - q/k projections fp8 DR: x8 (host-precast e4m3, DMA'd) broadcast into
    both slots, W as 32-scaled hi/lo e4m3 pair => half-cost projections.
  - Transposed PV: out[q, dims] = pt[k, q].T @ v_aug[k, dims]; per-head
    moving operand is only 65 cols ([v|1] / [1|v] with a shared ones
    column), halving PV PE cost vs streaming queries. Denominator lands
    in a per-partition column => normalize is per-partition
    tensor_scalar ops (no partition_broadcast, cheap reciprocal).
  - at (dims-major, needed as out-proj lhsT) recovered with a cheap PE
    transpose (128 rows bf16) + small PSUM->SBUF copy.
  - Emission-order splicing: proj/out-proj work chopped into small units
    and interleaved between score and PV matmuls so the in-order PE
    queue always has filler while exp (Act) catches up.
  - Band-limited causal affine_select (128 cols instead of row tail).
  - Out-proj copies split DVE/gpsimd; y written in one DMA per q-block.
"""

import sys

if "/opt/trn_rl_repo" not in sys.path:
    sys.path.insert(0, "/opt/trn_rl_repo")

from collections import deque

import numpy as np
import ml_dtypes

import concourse.bass as bass
import concourse.tile as tile
from concourse import bacc, mybir
from concourse.bass_utils import run_bass_kernel_spmd
from concourse.masks import make_identity

BF16 = mybir.dt.bfloat16
F8 = mybir.dt.float8e4
F32 = mybir.dt.float32
AF = mybir.ActivationFunctionType
DR = mybir.MatmulPerfMode.DoubleRow

N_EMBED = 1024
N_HEAD = 16
HEAD_DIM = 64
N_CORES = 8
HEADS_PER_CORE = N_HEAD // N_CORES          # 2
DCORE = HEADS_PER_CORE * HEAD_DIM           # 128
B = 2
S = 2048
QB = 512
KT = 128
DT = N_EMBED // 128                         # 8
SCALE = 1.0 / 8.0
H = HEADS_PER_CORE
VW = 192                                    # v_aug per-ktile row width
HV = 65                                     # per-head PV moving cols

QK_PROJ_FP8 = True
WSCALE = 32.0
N_WARM = 36


def build_program(seq=S):
    s_tot = B * seq
    n_qb = seq // QB
    n_kt = seq // KT
    kt_per_qb = QB // KT

    qk_scale = SCALE / (WSCALE * WSCALE) if QK_PROJ_FP8 else SCALE

    nc = bacc.Bacc("TRN2", target_bir_lowering=False, debug=False,
                   num_devices=N_CORES)

    xT = nc.dram_tensor("xT", [N_EMBED, s_tot], BF16, kind="ExternalInput")
    wv = nc.dram_tensor("wv", [N_EMBED, DCORE], BF16, kind="ExternalInput")
    bq = nc.dram_tensor("bq", [DCORE, 1], F32, kind="ExternalInput")
    bk = nc.dram_tensor("bk", [DCORE, 1], F32, kind="ExternalInput")
    wout = nc.dram_tensor("wout", [DCORE, N_EMBED], BF16, kind="ExternalInput")
    y = nc.dram_tensor("y", [s_tot, N_EMBED], BF16, kind="ExternalOutput")
    if QK_PROJ_FP8:
        xT8 = nc.dram_tensor("xT8", [N_EMBED, s_tot], F8, kind="ExternalInput")
        wq = nc.dram_tensor("wq", [N_EMBED, 2, DCORE], F8, kind="ExternalInput")
        wk = nc.dram_tensor("wk", [N_EMBED, 2, DCORE], F8, kind="ExternalInput")
    else:
        wq = nc.dram_tensor("wq", [N_EMBED, DCORE], BF16, kind="ExternalInput")
        wk = nc.dram_tensor("wk", [N_EMBED, DCORE], BF16, kind="ExternalInput")

    xT_r = xT.ap().rearrange("(t p) s -> p t s", p=128)

    with (
        tile.TileContext(nc) as tc,
        tc.tile_pool(name="singles", bufs=1) as singles,
        # PSUM banks (8): s_ps 2x2 = 4, attn_T 1x2 = 2, aux 2x1 = 2
        tc.tile_pool(name="s_ps", bufs=2, space="PSUM") as s_pool,
        tc.tile_pool(name="attnT_ps", bufs=1, space="PSUM") as attnT_pool,
        tc.tile_pool(name="aux_ps", bufs=2, space="PSUM") as aux_pool,
        tc.tile_pool(name="vstage", bufs=3) as vstage_pool,
        tc.tile_pool(name="pt_sb", bufs=10) as pt_pool,
        tc.tile_pool(name="rf_sb", bufs=4) as rf_pool,
        tc.tile_pool(name="atT_sb", bufs=4) as atT_pool,
        tc.tile_pool(name="at_sb", bufs=4) as at_pool,
        tc.tile_pool(name="y_sb", bufs=2) as ysb_pool,
    ):
        # ---- persistent SBUF tensors ----
        xT_sb = singles.tile([128, DT, s_tot], BF16)
        if QK_PROJ_FP8:
            x8_sb = singles.tile([128, DT, s_tot], F8)
            wq_sb = singles.tile([128, DT, 2, DCORE], F8)
            wk_sb = singles.tile([128, DT, 2, DCORE], F8)
        else:
            wq_sb = singles.tile([128, DT, DCORE], BF16)
            wk_sb = singles.tile([128, DT, DCORE], BF16)
        wv_sb = singles.tile([128, DT, DCORE], BF16)
        bq_sb = singles.tile([DCORE, 1], F32)
        bk_sb = singles.tile([DCORE, 1], F32)
        wout_sb = singles.tile([DCORE, N_EMBED], BF16)
        qT8 = singles.tile([DCORE, s_tot], F8)
        kT8 = singles.tile([DCORE, 2, s_tot], F8)   # [k_hi | k_lo] DR slots
        # per-ktile row: [v_h0(0:64) | 1@64 | gap | v_h1(96:160) | 1@160]
        # transpose dsts (0, 96) must be 32-aligned (XBAR tiles); both
        # head slices are [v(64) | ones] => PV out slot has dims at local
        # 0..63 and the softmax denominator at local col 64
        v_aug = singles.tile([128, B * n_kt, VW], BF16)
        ident_sb = singles.tile([128, 128], BF16)
        warm_sb = singles.tile([128, 128], BF16)

        # warm tile must be initialized (the correctness interpreter checks
        # finiteness); memset is cheap and unblocks the PE immediately
        nc.gpsimd.memset(warm_sb, 0.0)
        warm_ps = aux_pool.tile([128, 128], F32, tag="aux", name="warm")
        for _ in range(N_WARM):
            nc.tensor.matmul(warm_ps, lhsT=warm_sb, rhs=warm_sb[:, 0:128],
                             start=True, stop=True)

        # ---- input DMAs (SP queue) ----
        if QK_PROJ_FP8:
            nc.sync.dma_start(
                out=wq_sb, in_=wq.ap().rearrange("(t p) r h -> p t r h", p=128))
            nc.sync.dma_start(
                out=wk_sb, in_=wk.ap().rearrange("(t p) r h -> p t r h", p=128))
        else:
            nc.sync.dma_start(
                out=wq_sb, in_=wq.ap().rearrange("(t p) h -> p t h", p=128))
            nc.sync.dma_start(
                out=wk_sb, in_=wk.ap().rearrange("(t p) h -> p t h", p=128))
        nc.sync.dma_start(out=bq_sb, in_=bq.ap())
        nc.sync.dma_start(out=bk_sb, in_=bk.ap())
        if QK_PROJ_FP8:
            x8_r = xT8.ap().rearrange("(t p) s -> p t s", p=128)
            nc.sync.dma_start(out=x8_sb[:, :, 0:QB], in_=x8_r[:, :, 0:QB])
        nc.sync.dma_start(out=xT_sb[:, :, 0:QB], in_=xT_r[:, :, 0:QB])
        nc.sync.dma_start(out=wv_sb,
                          in_=wv.ap().rearrange("(t p) h -> p t h", p=128))
        for sb in range(1, s_tot // QB):
            sl = slice(sb * QB, (sb + 1) * QB)
            if QK_PROJ_FP8:
                nc.sync.dma_start(out=x8_sb[:, :, sl], in_=x8_r[:, :, sl])
            nc.sync.dma_start(out=xT_sb[:, :, sl], in_=xT_r[:, :, sl])
            if sb == min(2, s_tot // QB - 1):
                nc.sync.dma_start(out=wout_sb, in_=wout.ap())

        make_identity(nc, ident_sb)
        nc.gpsimd.memset(v_aug[:, :, 64:65], 1.0)    # h0 denom column
        nc.gpsimd.memset(v_aug[:, :, 160:161], 1.0)  # h1 denom column

        def proj_mm(ps, w_sb, sl, t):
            if QK_PROJ_FP8 and w_sb is not wv_sb:
                nc.tensor.matmul(
                    ps, lhsT=w_sb[:, t],
                    rhs=x8_sb[:, t, sl].unsqueeze(1).broadcast_to(
                        [128, 2, QB]),
                    start=(t == 0), stop=(t == DT - 1), perf_mode=DR)
            else:
                nc.tensor.matmul(ps, lhsT=w_sb[:, t], rhs=xT_sb[:, t, sl],
                                 start=(t == 0), stop=(t == DT - 1))

        def proj_gen(sb):
            """Emission units for projections of row-block sb."""
            sl = slice(sb * QB, (sb + 1) * QB)
            # --- q ---
            ps = aux_pool.tile([128, QB], F32, tag="aux", name="proj_q")
            for t in range(DT):
                proj_mm(ps, wq_sb, sl, t)
                if t == 3:
                    yield
            yield
            nc.vector.tensor_scalar_add(qT8[:, sl], ps, bq_sb)
            yield
            # --- k: hi + residual lo ---
            ps = aux_pool.tile([128, QB], F32, tag="aux", name="proj_k")
            for t in range(DT):
                proj_mm(ps, wk_sb, sl, t)
                if t == 3:
                    yield
            yield
            nc.vector.tensor_scalar_add(kT8[:, 0, sl], ps, bk_sb)
            nc.vector.scalar_tensor_tensor(
                out=kT8[:, 1, sl], in0=ps, scalar=bk_sb,
                op0=mybir.AluOpType.add, in1=kT8[:, 0, sl],
                op1=mybir.AluOpType.subtract)
            yield
            # --- v ---
            ps = aux_pool.tile([128, QB], F32, tag="aux", name="proj_v")
            for t in range(DT):
                proj_mm(ps, wv_sb, sl, t)
                if t == 3:
                    yield
            yield
            vstage = vstage_pool.tile([128, QB], BF16)
            nc.vector.tensor_copy(vstage, ps)
            yield
            for u in range(QB // 128):
                kt_gl = (QB // 128) * sb + u
                # h0 dims -> cols 0..63 (SP queue); h1 -> cols 96..159 (Act)
                nc.sync.dma_start_transpose(
                    out=v_aug[:, kt_gl, 0:HEAD_DIM],
                    in_=vstage[0:HEAD_DIM, u * 128:(u + 1) * 128])
                nc.scalar.dma_start_transpose(
                    out=v_aug[:, kt_gl, 96:160],
                    in_=vstage[HEAD_DIM:2 * HEAD_DIM,
                               u * 128:(u + 1) * 128])
                yield

        def outproj_gen(b_i, j, attn_T):
            """Normalize + out-projection units for q-block (b_i, j).
            All normalizes first: frees the (single-buffered) attn_T psum
            for the next block as early as possible."""
            ysb = ysb_pool.tile([128, QB // 128, N_EMBED], BF16, tag="ysb",
                                name="ysb")
            atTs = []
            for qt in range(QB // 128):
                rf = rf_pool.tile([128, 2], F32, tag="rf", name="rf")
                nc.vector.reciprocal_approx_fast(
                    rf[:, 0:1], attn_T[:, qt, 0, 64:65])
                nc.vector.reciprocal_approx_fast(
                    rf[:, 1:2], attn_T[:, qt, 1, 64:65])
                atT = atT_pool.tile([128, 128], BF16, tag="atT", name="atT")
                nc.vector.tensor_scalar_mul(
                    atT[:, 0:HEAD_DIM], attn_T[:, qt, 0, 0:HEAD_DIM],
                    rf[:, 0:1])
                nc.vector.tensor_scalar_mul(
                    atT[:, HEAD_DIM:128], attn_T[:, qt, 1, 0:HEAD_DIM],
                    rf[:, 1:2])
                atTs.append(atT)
                yield
            for qt in range(QB // 128):
                at_ps = aux_pool.tile([128, 128], BF16, tag="aux",
                                      name="at_ps")
                nc.tensor.transpose(at_ps, atTs[qt], ident_sb)
                at_sb = at_pool.tile([128, 128], BF16, tag="at", name="at")
                if qt % 2 == 0:
                    nc.vector.tensor_copy(at_sb, at_ps)
                else:
                    nc.scalar.copy(at_sb, at_ps)
                yield
                for u in range(N_EMBED // QB):
                    yp = aux_pool.tile([128, QB], F32, tag="aux", name="yp")
                    nc.tensor.matmul(yp, lhsT=at_sb,
                                     rhs=wout_sb[:, u * QB:(u + 1) * QB],
                                     start=True, stop=True)
                    dst = ysb[:, qt, u * QB:(u + 1) * QB]
                    if qt == 3 and u == 1:
                        nc.scalar.copy(dst, yp)   # small Act share
                    else:
                        nc.vector.tensor_copy(dst, yp)
                    yield
            row0 = b_i * seq + j * QB
            nc.sync.dma_start(
                out=y.ap()[row0:row0 + QB, :].rearrange(
                    "(q p) e -> p q e", p=128),
                in_=ysb)
            yield

        units = deque()          # outproj units
        punits = deque()         # proj units (ready earlier; prefer early)

        def _pump_from(q, n):
            while n > 0 and q:
                try:
                    next(q[0])
                    n -= 1
                except StopIteration:
                    q.popleft()
            return n

        def pump(n, prefer_proj=False):
            if prefer_proj:
                n = _pump_from(punits, n)
                _pump_from(units, n)
            else:
                n = _pump_from(units, n)
                _pump_from(punits, n)

        def attn_kloop(b_i, j):
            """Scores/exp/mask and deferred transposed-PV; diagonal k-tiles
            first; filler pumped between the score and PV matmuls."""
            q0 = b_i * seq + j * QB
            # 128-f32 slot per (qt, h): each matmul out stays inside one
            # PSUM bank; slots (qt0,qt1)->bank0, (qt2,qt3)->bank1
            attn_T = attnT_pool.tile([128, QB // 128, H, 128], F32,
                                     tag="attnT", name="attnT")
            kts = list(range(kt_per_qb * j, kt_per_qb * (j + 1))) + \
                list(range(0, kt_per_qb * j))
            n_pos = len(kts)

            def emit_pvt(kt, pt, d, pos):
                # PSUM start=True clears has_written for the WHOLE bank, so
                # exactly one matmul per bank per block may carry it; all
                # others rely on per-element first-write-stores semantics.
                kt_gl = b_i * n_kt + kt
                for qt in range(max(d, 0), QB // 128):
                    if j > 0:
                        stop = pos == n_pos - 1
                    else:
                        stop = d == qt
                    for h in range(H):
                        start = pos == 0 and h == 0 and qt in (0, 2)
                        nc.tensor.matmul(
                            attn_T[:, qt, h, 0:HV],
                            lhsT=pt[:, h, qt * 128:(qt + 1) * 128],
                            rhs=v_aug[:, kt_gl,
                                      96 * h:96 * h + HV],
                            start=start, stop=stop,
                            skip_group_check=True)

            pending = deque()    # PV deferred several k-tiles behind scores
            for pos, kt in enumerate(kts):
                ks = slice(b_i * seq + kt * 128, b_i * seq + kt * 128 + 128)
                d = kt - kt_per_qb * j
                off = 128 * d if d >= 0 else 0
                cols = QB - off
                s_ps = s_pool.tile([128, H, QB], F32, tag="s", name="s_ps")
                pt = pt_pool.tile([128, H, QB], BF16, tag="pt", name="pt")
                for h in range(H):
                    hsl = slice(HEAD_DIM * h, HEAD_DIM * (h + 1))
                    nc.tensor.matmul(
                        s_ps[:, h, off:],
                        lhsT=kT8[hsl, :, ks],
                        rhs=qT8[hsl, q0 + off:q0 + QB].unsqueeze(1)
                            .broadcast_to([HEAD_DIM, 2, cols]),
                        start=True, stop=True, perf_mode=DR)
                nc.scalar.activation(pt[:, :, off:], s_ps[:, :, off:],
                                     AF.Exp, scale=qk_scale)
                if d >= 0:  # mask only the 128-col diagonal band
                    nc.gpsimd.affine_select(
                        out=pt[:, :, off:off + 128],
                        in_=pt[:, :, off:off + 128],
                        compare_op=mybir.AluOpType.is_ge, fill=0.0,
                        base=0, channel_multiplier=-1,
                        pattern=[[0, H], [1, 128]])
                pump(2, prefer_proj=(pos < 6))
                pending.append((kt, pt, d, pos))
                if len(pending) > 6:
                    emit_pvt(*pending.popleft())
            while pending:
                emit_pvt(*pending.popleft())
            return attn_T

        # ---- schedule ----
        n_blocks = B * n_qb
        next_proj = 0
        for _ in range(3):
            if next_proj < n_blocks:
                punits.append(proj_gen(next_proj))
                next_proj += 1
        _pump_from(punits, 10 ** 9)   # frontload fully
        for b_i in range(B):
            for j in range(n_qb):
                if next_proj < n_blocks:
                    punits.append(proj_gen(next_proj))
                    next_proj += 1
                attn_T = attn_kloop(b_i, j)
                units.append(outproj_gen(b_i, j, attn_T))
        pump(10 ** 9)
        _pump_from(punits, 10 ** 9)   # drain the tail

    nc.compile()
    return nc


_CACHE = {}


def _get_program(seq=S):
    if seq not in _CACHE:
        _CACHE[seq] = build_program(seq)
    return _CACHE[seq]


def make_in_maps(x, W_qkv, b_qkv, seq=S):
    bf16 = ml_dtypes.bfloat16
    e4 = ml_dtypes.float8_e4m3
    s_tot = B * seq
    xT = np.ascontiguousarray(x.reshape(s_tot, N_EMBED).T).astype(bf16)
    xT8 = xT.astype(e4) if QK_PROJ_FP8 else None
    in_maps = []
    for c in range(N_CORES):
        csl = slice(DCORE * c, DCORE * (c + 1))
        m = {
            "xT": xT,
            "wv": np.ascontiguousarray(
                W_qkv[:, 2 * N_EMBED:][:, csl]).astype(bf16),
            "bq": np.ascontiguousarray(
                b_qkv[csl].reshape(DCORE, 1)).astype(np.float32),
            "bk": np.ascontiguousarray(
                b_qkv[N_EMBED:][csl].reshape(DCORE, 1)).astype(np.float32),
            "wout": None,  # filled by caller
        }
        if QK_PROJ_FP8:
            m["xT8"] = xT8
            for nm, off in (("wq", 0), ("wk", N_EMBED)):
                w = (WSCALE * W_qkv[:, off:][:, csl]).astype(np.float32)
                w_hi = w.astype(e4)
                w_lo = (w - w_hi.astype(np.float32)).astype(e4)
                m[nm] = np.ascontiguousarray(np.stack([w_hi, w_lo], axis=1))
            m["bq"] = m["bq"] * WSCALE
            m["bk"] = m["bk"] * WSCALE
        else:
            m["wq"] = np.ascontiguousarray(W_qkv[:, csl]).astype(bf16)
            m["wk"] = np.ascontiguousarray(
                W_qkv[:, N_EMBED:][:, csl]).astype(bf16)
        in_maps.append(m)
    return in_maps


def kernel(x, W_qkv, b_qkv, W_out, b_out):
    x = np.asarray(x, dtype=np.float32)
    W_qkv = np.asarray(W_qkv, dtype=np.float32)
    b_qkv = np.asarray(b_qkv, dtype=np.float32)
    W_out = np.asarray(W_out, dtype=np.float32)
    b_out = np.asarray(b_out, dtype=np.float32)

    nc = _get_program(S)
    in_maps = make_in_maps(x, W_qkv, b_qkv, S)
    bf16 = ml_dtypes.bfloat16
    for c in range(N_CORES):
        csl = slice(DCORE * c, DCORE * (c + 1))
        in_maps[c]["wout"] = np.ascontiguousarray(W_out[csl, :]).astype(bf16)

    res = run_bass_kernel_spmd(nc, in_maps, core_ids=list(range(N_CORES)))
    y = np.zeros((B * S, N_EMBED), dtype=np.float32)
    for r in res.results:
        y += r["y"].astype(np.float32)
    y += b_out[None, :] + b_qkv[2 * N_EMBED:] @ W_out
    return y.reshape(B, S, N_EMBED)


# revision 7
# speedup vs baseline: 1.1198x; 1.1198x over previous
"""Causal self-attention on 8 trn2 NeuronCores — all-fp8-DR projections,
fp8-DR scores, transposed-PV emission schedule.

Sharding: tensor-parallel over heads (2 heads/core).

Key techniques (v3):
  - ALL projections fp8 DoubleRow. q/k: x_hi e4m3 broadcast into both DR
    slots, W as 32-scaled [hi|lo] e4m3 residual pair (half cost). v: TWO
    DR matmuls (x_hi, then x_lo residual) against the wv [hi|lo] pair —
    ~bf16 accuracy at bf16 cost, but x ships to the device as two e4m3
    arrays (8MB instead of 12MB; DMA transfers are a serial resource).
  - The 32x weight scale on v cancels in the softmax normalize because
    the v_aug denominator ("ones") columns are memset to 32.0.
  - Scores in fp8e4 DR: kT8 carries [k_hi|k_lo], qT8 broadcast.
  - Transposed PV: out[q, dims] = pt[k, q].T @ v_aug[k, [v|ones]]; 65
    moving cols per head vs 128 queries => PV PE cost halved, denominator
    lands per-partition => normalize is two tiny per-partition ops, no
    partition_broadcast.
  - PSUM discipline: each (qt, h) PV slot is 128-f32 (bank-aligned, no
    bank crossing); start=True clears has_written BANK-wide, so exactly
    one PV matmul per bank per q-block carries start=True.
  - at (dims-major lhsT for the out-proj) via PE transpose + small copy.
  - Emission-order splicing: proj/out-proj units interleaved between
    score and PV matmuls so the in-order PE queue has filler while exp
    (Act) catches up; y written one DMA per q-block.
"""

import sys

if "/opt/trn_rl_repo" not in sys.path:
    sys.path.insert(0, "/opt/trn_rl_repo")

from collections import deque

import numpy as np
import ml_dtypes

import concourse.bass as bass
import concourse.tile as tile
from concourse import bacc, mybir
from concourse.bass_utils import run_bass_kernel_spmd
from concourse.masks import make_identity

BF16 = mybir.dt.bfloat16
F8 = mybir.dt.float8e4
F32 = mybir.dt.float32
AF = mybir.ActivationFunctionType
DR = mybir.MatmulPerfMode.DoubleRow

N_EMBED = 1024
N_HEAD = 16
HEAD_DIM = 64
N_CORES = 8
HEADS_PER_CORE = N_HEAD // N_CORES          # 2
DCORE = HEADS_PER_CORE * HEAD_DIM           # 128
B = 2
S = 2048
QB = 512
KT = 128
DT = N_EMBED // 128                         # 8
SCALE = 1.0 / 8.0
H = HEADS_PER_CORE
VW = 192                                    # v_aug per-ktile row width
HV = 65                                     # per-head PV moving cols

WSCALE = 32.0
N_WARM = 30


def build_program(seq=S):
    s_tot = B * seq
    n_qb = seq // QB
    n_kt = seq // KT
    kt_per_qb = QB // KT

    qk_scale = SCALE / (WSCALE * WSCALE)

    nc = bacc.Bacc("TRN2", target_bir_lowering=False, debug=False,
                   num_devices=N_CORES)

    x8h = nc.dram_tensor("x8h", [N_EMBED, s_tot], F8, kind="ExternalInput")
    x8l = nc.dram_tensor("x8l", [N_EMBED, s_tot], F8, kind="ExternalInput")
    wq = nc.dram_tensor("wq", [N_EMBED, 2, DCORE], F8, kind="ExternalInput")
    wk = nc.dram_tensor("wk", [N_EMBED, 2, DCORE], F8, kind="ExternalInput")
    wv = nc.dram_tensor("wv", [N_EMBED, 2, DCORE], F8, kind="ExternalInput")
    bq = nc.dram_tensor("bq", [DCORE, 1], F32, kind="ExternalInput")
    bk = nc.dram_tensor("bk", [DCORE, 1], F32, kind="ExternalInput")
    wout = nc.dram_tensor("wout", [DCORE, N_EMBED], BF16, kind="ExternalInput")
    y = nc.dram_tensor("y", [s_tot, N_EMBED], BF16, kind="ExternalOutput")

    with (
        tile.TileContext(nc) as tc,
        tc.tile_pool(name="singles", bufs=1) as singles,
        # PSUM banks (8): s_ps 2x2 = 4, attn_T 1x2 = 2, aux 2x1 = 2
        tc.tile_pool(name="s_ps", bufs=2, space="PSUM") as s_pool,
        tc.tile_pool(name="attnT_ps", bufs=1, space="PSUM") as attnT_pool,
        tc.tile_pool(name="aux_ps", bufs=2, space="PSUM") as aux_pool,
        tc.tile_pool(name="vstage", bufs=4) as vstage_pool,
        tc.tile_pool(name="pt_sb", bufs=12) as pt_pool,
        tc.tile_pool(name="rf_sb", bufs=4) as rf_pool,
        tc.tile_pool(name="atT_sb", bufs=5) as atT_pool,
        tc.tile_pool(name="at_sb", bufs=5) as at_pool,
        tc.tile_pool(name="y_sb", bufs=2) as ysb_pool,
    ):
        # ---- persistent SBUF tensors ----
        x8h_sb = singles.tile([128, DT, s_tot], F8)
        x8l_sb = singles.tile([128, DT, s_tot], F8)
        wq_sb = singles.tile([128, DT, 2, DCORE], F8)
        wk_sb = singles.tile([128, DT, 2, DCORE], F8)
        wv_sb = singles.tile([128, DT, 2, DCORE], F8)
        bq_sb = singles.tile([DCORE, 1], F32)
        bk_sb = singles.tile([DCORE, 1], F32)
        wout_sb = singles.tile([DCORE, N_EMBED], BF16)
        qT8 = singles.tile([DCORE, s_tot], F8)
        kT8 = singles.tile([DCORE, 2, s_tot], F8)   # [k_hi | k_lo] DR slots
        # per-ktile row: [v_h0(0:64) | 32@64 | gap | v_h1(96:160) | 32@160]
        # transpose dsts (0, 96) must be 32-aligned (XBAR tiles); both head
        # slices are [v(64) | ones*32]: PV out slot has dims at local 0..63
        # and (32x) softmax denominator at local col 64. The 32 cancels the
        # WSCALE on v.
        v_aug = singles.tile([128, B * n_kt, VW], BF16)
        ident_sb = singles.tile([128, 128], BF16)
        warm_sb = singles.tile([128, 128], BF16)

        # warm tile must be initialized (the interpreter checks finiteness);
        # memset is cheap and unblocks the PE immediately
        nc.gpsimd.memset(warm_sb, 0.0)
        warm_ps = aux_pool.tile([128, 128], F32, tag="aux", name="warm")
        for _ in range(N_WARM):
            nc.tensor.matmul(warm_ps, lhsT=warm_sb, rhs=warm_sb[:, 0:128],
                             start=True, stop=True)

        # ---- input DMAs (SP queue); x_hi/x_lo pair-streamed per block ----
        def wrearr(w):
            return w.ap().rearrange("(t p) r h -> p t r h", p=128)

        nc.sync.dma_start(out=wq_sb, in_=wrearr(wq))
        nc.sync.dma_start(out=wk_sb, in_=wrearr(wk))
        nc.sync.dma_start(out=bq_sb, in_=bq.ap())
        nc.sync.dma_start(out=bk_sb, in_=bk.ap())
        x8h_r = x8h.ap().rearrange("(t p) s -> p t s", p=128)
        x8l_r = x8l.ap().rearrange("(t p) s -> p t s", p=128)
        for sb in range(s_tot // QB):
            sl = slice(sb * QB, (sb + 1) * QB)
            nc.sync.dma_start(out=x8h_sb[:, :, sl], in_=x8h_r[:, :, sl])
            nc.sync.dma_start(out=x8l_sb[:, :, sl], in_=x8l_r[:, :, sl])
            if sb == 0:
                nc.sync.dma_start(out=wv_sb, in_=wrearr(wv))
            elif sb == 1:
                nc.sync.dma_start(out=wout_sb, in_=wout.ap())

        make_identity(nc, ident_sb)
        nc.gpsimd.memset(v_aug[:, :, 64:65], WSCALE)    # h0 denom column
        nc.gpsimd.memset(v_aug[:, :, 160:161], WSCALE)  # h1 denom column

        def proj_gen(sb):
            """Emission units for projections of row-block sb."""
            sl = slice(sb * QB, (sb + 1) * QB)

            def xbc(xs, t):
                return xs[:, t, sl].unsqueeze(1).broadcast_to([128, 2, QB])

            # --- q ---
            ps = aux_pool.tile([128, QB], F32, tag="aux", name="proj_q")
            for t in range(DT):
                nc.tensor.matmul(ps, lhsT=wq_sb[:, t], rhs=xbc(x8h_sb, t),
                                 start=(t == 0), stop=(t == DT - 1),
                                 perf_mode=DR)
                if t == 3:
                    yield
            yield
            nc.vector.tensor_scalar_add(qT8[:, sl], ps, bq_sb)
            yield
            # --- k: hi + residual lo ---
            ps = aux_pool.tile([128, QB], F32, tag="aux", name="proj_k")
            for t in range(DT):
                nc.tensor.matmul(ps, lhsT=wk_sb[:, t], rhs=xbc(x8h_sb, t),
                                 start=(t == 0), stop=(t == DT - 1),
                                 perf_mode=DR)
                if t == 3:
                    yield
            yield
            nc.vector.tensor_scalar_add(kT8[:, 0, sl], ps, bk_sb)
            nc.vector.scalar_tensor_tensor(
                out=kT8[:, 1, sl], in0=ps, scalar=bk_sb,
                op0=mybir.AluOpType.add, in1=kT8[:, 0, sl],
                op1=mybir.AluOpType.subtract)
            yield
            # --- v: (x_hi + x_lo) against the wv [hi|lo] pair ---
            ps = aux_pool.tile([128, QB], F32, tag="aux", name="proj_v")
            for t in range(DT):
                nc.tensor.matmul(ps, lhsT=wv_sb[:, t], rhs=xbc(x8h_sb, t),
                                 start=(t == 0), stop=False, perf_mode=DR)
                nc.tensor.matmul(ps, lhsT=wv_sb[:, t], rhs=xbc(x8l_sb, t),
                                 start=False, stop=(t == DT - 1),
                                 perf_mode=DR)
                if t % 3 == 2:
                    yield
            yield
            vstage = vstage_pool.tile([128, QB], BF16)
            nc.vector.tensor_copy(vstage, ps)
            yield
            for u in range(QB // 128):
                kt_gl = (QB // 128) * sb + u
                # h0 dims -> cols 0..63 (SP queue); h1 -> cols 96..159 (Act)
                nc.sync.dma_start_transpose(
                    out=v_aug[:, kt_gl, 0:HEAD_DIM],
                    in_=vstage[0:HEAD_DIM, u * 128:(u + 1) * 128])
                nc.scalar.dma_start_transpose(
                    out=v_aug[:, kt_gl, 96:96 + HEAD_DIM],
                    in_=vstage[HEAD_DIM:2 * HEAD_DIM,
                               u * 128:(u + 1) * 128])
                yield

        def outproj_gen(b_i, j, attn_T):
            """Normalize + out-projection units for q-block (b_i, j).
            All normalizes first: frees the (single-buffered) attn_T psum
            for the next block as early as possible."""
            ysb = ysb_pool.tile([128, QB // 128, N_EMBED], BF16, tag="ysb",
                                name="ysb")
            atTs = []
            for qt in range(QB // 128):
                rf = rf_pool.tile([128, 2], F32, tag="rf", name="rf")
                nc.vector.reciprocal_approx_fast(
                    rf[:, 0:1], attn_T[:, qt, 0, 64:65])
                nc.vector.reciprocal_approx_fast(
                    rf[:, 1:2], attn_T[:, qt, 1, 64:65])
                atT = atT_pool.tile([128, 128], BF16, tag="atT", name="atT")
                nc.vector.tensor_scalar_mul(
                    atT[:, 0:HEAD_DIM], attn_T[:, qt, 0, 0:HEAD_DIM],
                    rf[:, 0:1])
                nc.vector.tensor_scalar_mul(
                    atT[:, HEAD_DIM:128], attn_T[:, qt, 1, 0:HEAD_DIM],
                    rf[:, 1:2])
                atTs.append(atT)
                yield
            for qt in range(QB // 128):
                at_ps = aux_pool.tile([128, 128], BF16, tag="aux",
                                      name="at_ps")
                nc.tensor.transpose(at_ps, atTs[qt], ident_sb)
                at_sb = at_pool.tile([128, 128], BF16, tag="at", name="at")
                nc.vector.tensor_copy(at_sb, at_ps)
                yield
                for u in range(N_EMBED // QB):
                    yp = aux_pool.tile([128, QB], F32, tag="aux", name="yp")
                    nc.tensor.matmul(yp, lhsT=at_sb,
                                     rhs=wout_sb[:, u * QB:(u + 1) * QB],
                                     start=True, stop=True)
                    dst = ysb[:, qt, u * QB:(u + 1) * QB]
                    if u == 1 and qt % 2 == 1:
                        nc.scalar.copy(dst, yp)   # Act share (16 of 64)
                    else:
                        nc.vector.tensor_copy(dst, yp)
                    yield
            row0 = b_i * seq + j * QB
            nc.sync.dma_start(
                out=y.ap()[row0:row0 + QB, :].rearrange(
                    "(q p) e -> p q e", p=128),
                in_=ysb)
            yield

        units = deque()          # outproj units
        punits = deque()         # proj units (ready earlier; prefer early)

        def _pump_from(q, n):
            while n > 0 and q:
                try:
                    next(q[0])
                    n -= 1
                except StopIteration:
                    q.popleft()
            return n

        def pump(n, prefer_proj=False):
            if prefer_proj:
                n = _pump_from(punits, n)
                _pump_from(units, n)
            else:
                n = _pump_from(units, n)
                _pump_from(punits, n)

        def attn_kloop(b_i, j):
            """Scores/exp/mask and deferred transposed-PV; diagonal k-tiles
            first; filler pumped between the score and PV matmuls."""
            q0 = b_i * seq + j * QB
            # 128-f32 slot per (qt, h): each matmul out stays inside one
            # PSUM bank; slots (qt0,qt1)->bank0, (qt2,qt3)->bank1
            attn_T = attnT_pool.tile([128, QB // 128, H, 128], F32,
                                     tag="attnT", name="attnT")
            kts = list(range(kt_per_qb * j, kt_per_qb * (j + 1))) + \
                list(range(0, kt_per_qb * j))
            n_pos = len(kts)

            def emit_pvt(kt, pt, d, pos):
                # PSUM start=True clears has_written for the WHOLE bank, so
                # exactly one matmul per bank per block may carry it; all
                # others rely on per-element first-write-stores semantics.
                kt_gl = b_i * n_kt + kt
                for qt in range(max(d, 0), QB // 128):
                    if j > 0:
                        stop = pos == n_pos - 1
                    else:
                        stop = d == qt
                    for h in range(H):
                        start = pos == 0 and h == 0 and qt in (0, 2)
                        nc.tensor.matmul(
                            attn_T[:, qt, h, 0:HV],
                            lhsT=pt[:, h, qt * 128:(qt + 1) * 128],
                            rhs=v_aug[:, kt_gl, 96 * h:96 * h + HV],
                            start=start, stop=stop,
                            skip_group_check=True)

            pending = deque()    # PV deferred several k-tiles behind scores
            for pos, kt in enumerate(kts):
                ks = slice(b_i * seq + kt * 128, b_i * seq + kt * 128 + 128)
                d = kt - kt_per_qb * j
                off = 128 * d if d >= 0 else 0
                cols = QB - off
                s_ps = s_pool.tile([128, H, QB], F32, tag="s", name="s_ps")
                pt = pt_pool.tile([128, H, QB], BF16, tag="pt", name="pt")
                for h in range(H):
                    hsl = slice(HEAD_DIM * h, HEAD_DIM * (h + 1))
                    nc.tensor.matmul(
                        s_ps[:, h, off:],
                        lhsT=kT8[hsl, :, ks],
                        rhs=qT8[hsl, q0 + off:q0 + QB].unsqueeze(1)
                            .broadcast_to([HEAD_DIM, 2, cols]),
                        start=True, stop=True, perf_mode=DR)
                nc.scalar.activation(pt[:, :, off:], s_ps[:, :, off:],
                                     AF.Exp, scale=qk_scale)
                if d >= 0:  # mask only the 128-col diagonal band
                    nc.gpsimd.affine_select(
                        out=pt[:, :, off:off + 128],
                        in_=pt[:, :, off:off + 128],
                        compare_op=mybir.AluOpType.is_ge, fill=0.0,
                        base=0, channel_multiplier=-1,
                        pattern=[[0, H], [1, 128]])
                pump(2, prefer_proj=(pos < 6))
                pending.append((kt, pt, d, pos))
                if len(pending) > 6:
                    emit_pvt(*pending.popleft())
            while pending:
                emit_pvt(*pending.popleft())
            return attn_T

        # ---- schedule ----
        n_blocks = B * n_qb
        next_proj = 0
        for _ in range(3):
            if next_proj < n_blocks:
                punits.append(proj_gen(next_proj))
                next_proj += 1
        _pump_from(punits, 10 ** 9)   # frontload fully
        for b_i in range(B):
            for j in range(n_qb):
                if next_proj < n_blocks:
                    punits.append(proj_gen(next_proj))
                    next_proj += 1
                attn_T = attn_kloop(b_i, j)
                units.append(outproj_gen(b_i, j, attn_T))
        pump(10 ** 9)
        _pump_from(punits, 10 ** 9)   # drain the tail

    nc.compile()
    return nc


_CACHE = {}


def _get_program(seq=S):
    if seq not in _CACHE:
        _CACHE[seq] = build_program(seq)
    return _CACHE[seq]


def make_in_maps(x, W_qkv, b_qkv, seq=S):
    e4 = ml_dtypes.float8_e4m3
    s_tot = B * seq
    xT = np.ascontiguousarray(
        x.reshape(s_tot, N_EMBED).T).astype(np.float32)
    x8h = xT.astype(e4)
    x8l = (xT - x8h.astype(np.float32)).astype(e4)
    in_maps = []
    for c in range(N_CORES):
        csl = slice(DCORE * c, DCORE * (c + 1))
        m = {
            "x8h": x8h,
            "x8l": x8l,
            "bq": np.ascontiguousarray(
                b_qkv[csl].reshape(DCORE, 1)).astype(np.float32) * WSCALE,
            "bk": np.ascontiguousarray(
                b_qkv[N_EMBED:][csl].reshape(DCORE, 1)).astype(
                    np.float32) * WSCALE,
            "wout": None,  # filled by caller
        }
        for nm, off in (("wq", 0), ("wk", N_EMBED), ("wv", 2 * N_EMBED)):
            w = (WSCALE * W_qkv[:, off:][:, csl]).astype(np.float32)
            w_hi = w.astype(e4)
            w_lo = (w - w_hi.astype(np.float32)).astype(e4)
            m[nm] = np.ascontiguousarray(np.stack([w_hi, w_lo], axis=1))
        in_maps.append(m)
    return in_maps


def kernel(x, W_qkv, b_qkv, W_out, b_out):
    x = np.asarray(x, dtype=np.float32)
    W_qkv = np.asarray(W_qkv, dtype=np.float32)
    b_qkv = np.asarray(b_qkv, dtype=np.float32)
    W_out = np.asarray(W_out, dtype=np.float32)
    b_out = np.asarray(b_out, dtype=np.float32)

    nc = _get_program(S)
    in_maps = make_in_maps(x, W_qkv, b_qkv, S)
    bf16 = ml_dtypes.bfloat16
    for c in range(N_CORES):
        csl = slice(DCORE * c, DCORE * (c + 1))
        in_maps[c]["wout"] = np.ascontiguousarray(W_out[csl, :]).astype(bf16)

    res = run_bass_kernel_spmd(nc, in_maps, core_ids=list(range(N_CORES)))
    y = np.zeros((B * S, N_EMBED), dtype=np.float32)
    for r in res.results:
        y += r["y"].astype(np.float32)
    y += b_out[None, :] + b_qkv[2 * N_EMBED:] @ W_out
    return y.reshape(B, S, N_EMBED)


# revision 9
# speedup vs baseline: 1.5274x; 1.3640x over previous
"""Causal self-attention on 8 trn2 NeuronCores — all-fp8-DR projections,
fp8-DR scores, transposed-PV emission schedule.

Sharding: tensor-parallel over heads (2 heads/core).

Key techniques (v3):
  - ALL projections fp8 DoubleRow. q/k: x_hi e4m3 broadcast into both DR
    slots, W as 32-scaled [hi|lo] e4m3 residual pair (half cost). v: TWO
    DR matmuls (x_hi, then x_lo residual) against the wv [hi|lo] pair —
    ~bf16 accuracy at bf16 cost, but x ships to the device as two e4m3
    arrays (8MB instead of 12MB; DMA transfers are a serial resource).
  - The 32x weight scale on v cancels in the softmax normalize because
    the v_aug denominator ("ones") columns are memset to 32.0.
  - Scores in fp8e4 DR: kT8 carries [k_hi|k_lo], qT8 broadcast.
  - Transposed PV: out[q, dims] = pt[k, q].T @ v_aug[k, [v|ones]]; 65
    moving cols per head vs 128 queries => PV PE cost halved, denominator
    lands per-partition => normalize is two tiny per-partition ops, no
    partition_broadcast.
  - PSUM discipline: each (qt, h) PV slot is 128-f32 (bank-aligned, no
    bank crossing); start=True clears has_written BANK-wide, so exactly
    one PV matmul per bank per q-block carries start=True.
  - at (dims-major lhsT for the out-proj) via PE transpose + small copy.
  - Emission-order splicing: proj/out-proj units interleaved between
    score and PV matmuls so the in-order PE queue has filler while exp
    (Act) catches up; y written one DMA per q-block.
"""

import sys

if "/opt/trn_rl_repo" not in sys.path:
    sys.path.insert(0, "/opt/trn_rl_repo")

from collections import deque

import numpy as np
import ml_dtypes

import concourse.bass as bass
import concourse.tile as tile
from concourse import bacc, mybir
from concourse.bass_utils import run_bass_kernel_spmd
from concourse.masks import make_identity

BF16 = mybir.dt.bfloat16
F8 = mybir.dt.float8e4
F32 = mybir.dt.float32
AF = mybir.ActivationFunctionType
DR = mybir.MatmulPerfMode.DoubleRow

N_EMBED = 1024
N_HEAD = 16
HEAD_DIM = 64
N_CORES = 8
HEADS_PER_CORE = N_HEAD // N_CORES          # 2
DCORE = HEADS_PER_CORE * HEAD_DIM           # 128
B = 2
S = 2048
QB = 512
KT = 128
DT = N_EMBED // 128                         # 8
SCALE = 1.0 / 8.0
H = HEADS_PER_CORE
VW = 160                                    # v_aug per-ktile row width (32-mult)
HV = 65                                     # per-head PV moving cols

WSCALE = 32.0
N_WARM = 30


def build_program(seq=S):
    s_tot = B * seq
    n_qb = seq // QB
    n_kt = seq // KT
    kt_per_qb = QB // KT

    qk_scale = SCALE / (WSCALE * WSCALE)

    nc = bacc.Bacc("TRN2", target_bir_lowering=False, debug=False,
                   num_devices=N_CORES)

    x8h = nc.dram_tensor("x8h", [N_EMBED, s_tot], F8, kind="ExternalInput")
    x8l = nc.dram_tensor("x8l", [N_EMBED, s_tot], F8, kind="ExternalInput")
    wq = nc.dram_tensor("wq", [N_EMBED, 2, DCORE], F8, kind="ExternalInput")
    wk = nc.dram_tensor("wk", [N_EMBED, 2, DCORE], F8, kind="ExternalInput")
    wv = nc.dram_tensor("wv", [N_EMBED, 2, DCORE], F8, kind="ExternalInput")
    bq = nc.dram_tensor("bq", [DCORE, 1], F32, kind="ExternalInput")
    bk = nc.dram_tensor("bk", [DCORE, 1], F32, kind="ExternalInput")
    wout = nc.dram_tensor("wout", [DCORE, N_EMBED], BF16, kind="ExternalInput")
    y = nc.dram_tensor("y", [s_tot, N_EMBED], BF16, kind="ExternalOutput")

    with (
        tile.TileContext(nc) as tc,
        tc.tile_pool(name="singles", bufs=1) as singles,
        # PSUM banks (8): s_ps 2x2 = 4, attn_T 1x2 = 2, aux 2x1 = 2
        tc.tile_pool(name="s_ps", bufs=2, space="PSUM") as s_pool,
        tc.tile_pool(name="attnT_ps", bufs=1, space="PSUM") as attnT_pool,
        tc.tile_pool(name="aux_ps", bufs=2, space="PSUM") as aux_pool,
        tc.tile_pool(name="vstage", bufs=4) as vstage_pool,
        tc.tile_pool(name="pt_sb", bufs=12) as pt_pool,
        tc.tile_pool(name="rf_sb", bufs=4) as rf_pool,
        tc.tile_pool(name="atT_sb", bufs=5) as atT_pool,
        tc.tile_pool(name="at_sb", bufs=5) as at_pool,
        tc.tile_pool(name="y_sb", bufs=2) as ysb_pool,
    ):
        # ---- persistent SBUF tensors ----
        x8h_sb = singles.tile([128, DT, s_tot], F8)
        x8l_sb = singles.tile([128, DT, s_tot], F8)
        wq_sb = singles.tile([128, DT, 2, DCORE], F8)
        wk_sb = singles.tile([128, DT, 2, DCORE], F8)
        wv_sb = singles.tile([128, DT, 2, DCORE], F8)
        bq_sb = singles.tile([DCORE, 1], F32)
        bk_sb = singles.tile([DCORE, 1], F32)
        wout_sb = singles.tile([DCORE, N_EMBED], BF16)
        qT8 = singles.tile([DCORE, s_tot], F8)
        kT8 = singles.tile([DCORE, 2, s_tot], F8)   # [k_hi | k_lo] DR slots
        # per-ktile row: [v_h0(0:64) | v_h1(64:128) | 32@128]: one
        # contiguous 128-col DMA-transpose destination per 128-token chunk.
        # h1's PV moving slice [v1|32] is contiguous (cols 64..128); h0
        # takes a separate (nearly free) 1-col denominator matmul. The 32
        # cancels the WSCALE on v inside the softmax normalize.
        v_aug = singles.tile([128, B * n_kt, VW], BF16)
        ident_sb = singles.tile([128, 128], BF16)
        warm_sb = singles.tile([128, 128], BF16)

        # warm tile must be initialized (the interpreter checks finiteness);
        # memset is cheap and unblocks the PE immediately
        nc.gpsimd.memset(warm_sb, 0.0)
        warm_ps = aux_pool.tile([128, 128], F32, tag="aux", name="warm")
        for _ in range(N_WARM):
            nc.tensor.matmul(warm_ps, lhsT=warm_sb, rhs=warm_sb[:, 0:128],
                             start=True, stop=True)

        # ---- input DMAs (SP queue); x_hi/x_lo pair-streamed per block ----
        def wrearr(w):
            return w.ap().rearrange("(t p) r h -> p t r h", p=128)

        nc.sync.dma_start(out=wq_sb, in_=wrearr(wq))
        nc.sync.dma_start(out=wk_sb, in_=wrearr(wk))
        nc.sync.dma_start(out=bq_sb, in_=bq.ap())
        nc.sync.dma_start(out=bk_sb, in_=bk.ap())
        x8h_r = x8h.ap().rearrange("(t p) s -> p t s", p=128)
        x8l_r = x8l.ap().rearrange("(t p) s -> p t s", p=128)
        for sb in range(s_tot // QB):
            sl = slice(sb * QB, (sb + 1) * QB)
            nc.sync.dma_start(out=x8h_sb[:, :, sl], in_=x8h_r[:, :, sl])
            nc.sync.dma_start(out=x8l_sb[:, :, sl], in_=x8l_r[:, :, sl])
            if sb == 0:
                nc.sync.dma_start(out=wv_sb, in_=wrearr(wv))
            elif sb == 1:
                nc.sync.dma_start(out=wout_sb, in_=wout.ap())

        make_identity(nc, ident_sb)
        nc.gpsimd.memset(v_aug[:, :, 128:129], WSCALE)  # shared denom column

        def proj_gen(sb):
            """Emission units for projections of row-block sb."""
            sl = slice(sb * QB, (sb + 1) * QB)

            def xbc(xs, t):
                return xs[:, t, sl].unsqueeze(1).broadcast_to([128, 2, QB])

            # --- q ---
            ps = aux_pool.tile([128, QB], F32, tag="aux", name="proj_q")
            for t in range(DT):
                nc.tensor.matmul(ps, lhsT=wq_sb[:, t], rhs=xbc(x8h_sb, t),
                                 start=(t == 0), stop=(t == DT - 1),
                                 perf_mode=DR)
                if t == 3:
                    yield
            yield
            nc.vector.tensor_scalar_add(qT8[:, sl], ps, bq_sb)
            yield
            # --- k: hi + residual lo ---
            ps = aux_pool.tile([128, QB], F32, tag="aux", name="proj_k")
            for t in range(DT):
                nc.tensor.matmul(ps, lhsT=wk_sb[:, t], rhs=xbc(x8h_sb, t),
                                 start=(t == 0), stop=(t == DT - 1),
                                 perf_mode=DR)
                if t == 3:
                    yield
            yield
            nc.vector.tensor_scalar_add(kT8[:, 0, sl], ps, bk_sb)
            nc.vector.scalar_tensor_tensor(
                out=kT8[:, 1, sl], in0=ps, scalar=bk_sb,
                op0=mybir.AluOpType.add, in1=kT8[:, 0, sl],
                op1=mybir.AluOpType.subtract)
            yield
            # --- v: (x_hi + x_lo) against the wv [hi|lo] pair ---
            ps = aux_pool.tile([128, QB], F32, tag="aux", name="proj_v")
            for t in range(DT):
                nc.tensor.matmul(ps, lhsT=wv_sb[:, t], rhs=xbc(x8h_sb, t),
                                 start=(t == 0), stop=False, perf_mode=DR)
                nc.tensor.matmul(ps, lhsT=wv_sb[:, t], rhs=xbc(x8l_sb, t),
                                 start=False, stop=(t == DT - 1),
                                 perf_mode=DR)
                if t % 3 == 2:
                    yield
            yield
            vstage = vstage_pool.tile([128, QB], BF16)
            nc.vector.tensor_copy(vstage, ps)
            yield
            for u in range(QB // 128):
                kt_gl = (QB // 128) * sb + u
                nc.sync.dma_start_transpose(
                    out=v_aug[:, kt_gl, 0:128],
                    in_=vstage[:, u * 128:(u + 1) * 128])
                yield

        def outproj_gen(b_i, j, attn_T, tail=False):
            """Normalize + out-projection units for q-block (b_i, j).
            All normalizes first: frees the (single-buffered) attn_T psum
            for the next block as early as possible. tail=True streams the
            y DMA per 128-row chunk to shorten the kernel epilogue."""
            ysb = ysb_pool.tile([128, QB // 128, N_EMBED], BF16, tag="ysb",
                                name="ysb")
            atTs = []
            for qt in range(QB // 128):
                rf = rf_pool.tile([128, 2], F32, tag="rf", name="rf")
                nc.vector.reciprocal_approx_fast(
                    rf, attn_T[:, qt, :, 64:65])
                atT = atT_pool.tile([128, 128], BF16, tag="atT", name="atT")
                nc.vector.tensor_scalar_mul(
                    atT[:, 0:HEAD_DIM], attn_T[:, qt, 0, 0:HEAD_DIM],
                    rf[:, 0:1])
                nc.vector.tensor_scalar_mul(
                    atT[:, HEAD_DIM:128], attn_T[:, qt, 1, 0:HEAD_DIM],
                    rf[:, 1:2])
                atTs.append(atT)
                yield
            for qt in range(QB // 128):
                at_ps = aux_pool.tile([128, 128], BF16, tag="aux",
                                      name="at_ps")
                nc.tensor.transpose(at_ps, atTs[qt], ident_sb)
                at_sb = at_pool.tile([128, 128], BF16, tag="at", name="at")
                nc.vector.tensor_copy(at_sb, at_ps)
                yield
                for u in range(N_EMBED // QB):
                    yp = aux_pool.tile([128, QB], F32, tag="aux", name="yp")
                    nc.tensor.matmul(yp, lhsT=at_sb,
                                     rhs=wout_sb[:, u * QB:(u + 1) * QB],
                                     start=True, stop=True)
                    dst = ysb[:, qt, u * QB:(u + 1) * QB]
                    nc.vector.tensor_copy(dst, yp)
                    yield
                if tail:
                    r0 = b_i * seq + j * QB + qt * 128
                    nc.sync.dma_start(out=y.ap()[r0:r0 + 128, :],
                                      in_=ysb[:, qt])
                    yield
            if not tail:
                row0 = b_i * seq + j * QB
                nc.sync.dma_start(
                    out=y.ap()[row0:row0 + QB, :].rearrange(
                        "(q p) e -> p q e", p=128),
                    in_=ysb)
                yield

        units = deque()          # outproj units
        punits = deque()         # proj units (ready earlier; prefer early)

        def _pump_from(q, n):
            while n > 0 and q:
                try:
                    next(q[0])
                    n -= 1
                except StopIteration:
                    q.popleft()
            return n

        def pump(n, prefer_proj=False):
            if prefer_proj:
                n = _pump_from(punits, n)
                _pump_from(units, n)
            else:
                n = _pump_from(units, n)
                _pump_from(punits, n)

        def attn_kloop(b_i, j):
            """Scores/exp/mask and deferred transposed-PV; diagonal k-tiles
            first; filler pumped between the score and PV matmuls."""
            q0 = b_i * seq + j * QB
            # 128-f32 slot per (qt, h): each matmul out stays inside one
            # PSUM bank; slots (qt0,qt1)->bank0, (qt2,qt3)->bank1
            attn_T = attnT_pool.tile([128, QB // 128, H, 128], F32,
                                     tag="attnT", name="attnT")
            kts = list(range(kt_per_qb * j, kt_per_qb * (j + 1))) + \
                list(range(0, kt_per_qb * j))
            n_pos = len(kts)

            def emit_pvt(kt, pt, d, pos):
                # PSUM start=True clears has_written for the WHOLE bank, so
                # exactly one matmul per bank per block may carry it; all
                # others rely on per-element first-write-stores semantics.
                kt_gl = b_i * n_kt + kt
                for qt in range(max(d, 0), QB // 128):
                    if j > 0:
                        stop = pos == n_pos - 1
                    else:
                        stop = d == qt
                    # h0: dims (cols 0..63) + 1-col denominator (col 128)
                    start = pos == 0 and qt in (0, 2)
                    nc.tensor.matmul(
                        attn_T[:, qt, 0, 0:HEAD_DIM],
                        lhsT=pt[:, 0, qt * 128:(qt + 1) * 128],
                        rhs=v_aug[:, kt_gl, 0:HEAD_DIM],
                        start=start, stop=stop, skip_group_check=True)
                    nc.tensor.matmul(
                        attn_T[:, qt, 0, HEAD_DIM:HV],
                        lhsT=pt[:, 0, qt * 128:(qt + 1) * 128],
                        rhs=v_aug[:, kt_gl, 128:129],
                        start=False, stop=stop, skip_group_check=True)
                    # h1: contiguous [v1 | 32] => dims + denominator in one
                    nc.tensor.matmul(
                        attn_T[:, qt, 1, 0:HV],
                        lhsT=pt[:, 1, qt * 128:(qt + 1) * 128],
                        rhs=v_aug[:, kt_gl, HEAD_DIM:HV + HEAD_DIM],
                        start=False, stop=stop, skip_group_check=True)

            pending = deque()    # PV deferred several k-tiles behind scores
            for pos, kt in enumerate(kts):
                ks = slice(b_i * seq + kt * 128, b_i * seq + kt * 128 + 128)
                d = kt - kt_per_qb * j
                off = 128 * d if d >= 0 else 0
                cols = QB - off
                s_ps = s_pool.tile([128, H, QB], F32, tag="s", name="s_ps")
                pt = pt_pool.tile([128, H, QB], BF16, tag="pt", name="pt")
                for h in range(H):
                    hsl = slice(HEAD_DIM * h, HEAD_DIM * (h + 1))
                    nc.tensor.matmul(
                        s_ps[:, h, off:],
                        lhsT=kT8[hsl, :, ks],
                        rhs=qT8[hsl, q0 + off:q0 + QB].unsqueeze(1)
                            .broadcast_to([HEAD_DIM, 2, cols]),
                        start=True, stop=True, perf_mode=DR)
                nc.scalar.activation(pt[:, :, off:], s_ps[:, :, off:],
                                     AF.Exp, scale=qk_scale)
                if d >= 0:  # mask only the 128-col diagonal band
                    nc.gpsimd.affine_select(
                        out=pt[:, :, off:off + 128],
                        in_=pt[:, :, off:off + 128],
                        compare_op=mybir.AluOpType.is_ge, fill=0.0,
                        base=0, channel_multiplier=-1,
                        pattern=[[0, H], [1, 128]])
                pump(2, prefer_proj=(pos < 6))
                pending.append((kt, pt, d, pos))
                if len(pending) > 6:
                    emit_pvt(*pending.popleft())
            while pending:
                emit_pvt(*pending.popleft())
            return attn_T

        # ---- schedule ----
        n_blocks = B * n_qb
        next_proj = 0
        for _ in range(3):
            if next_proj < n_blocks:
                punits.append(proj_gen(next_proj))
                next_proj += 1
        _pump_from(punits, 10 ** 9)   # frontload fully
        for b_i in range(B):
            for j in range(n_qb):
                if next_proj < n_blocks:
                    punits.append(proj_gen(next_proj))
                    next_proj += 1
                attn_T = attn_kloop(b_i, j)
                tail = b_i == B - 1 and j == n_qb - 1
                units.append(outproj_gen(b_i, j, attn_T, tail=tail))
        pump(10 ** 9)
        _pump_from(punits, 10 ** 9)   # drain the tail

    nc.compile()
    return nc


_CACHE = {}


def _get_program(seq=S):
    if seq not in _CACHE:
        _CACHE[seq] = build_program(seq)
    return _CACHE[seq]


def make_in_maps(x, W_qkv, b_qkv, seq=S):
    e4 = ml_dtypes.float8_e4m3
    s_tot = B * seq
    xT = np.ascontiguousarray(
        x.reshape(s_tot, N_EMBED).T).astype(np.float32)
    x8h = xT.astype(e4)
    x8l = (xT - x8h.astype(np.float32)).astype(e4)
    in_maps = []
    for c in range(N_CORES):
        csl = slice(DCORE * c, DCORE * (c + 1))
        m = {
            "x8h": x8h,
            "x8l": x8l,
            "bq": np.ascontiguousarray(
                b_qkv[csl].reshape(DCORE, 1)).astype(np.float32) * WSCALE,
            "bk": np.ascontiguousarray(
                b_qkv[N_EMBED:][csl].reshape(DCORE, 1)).astype(
                    np.float32) * WSCALE,
            "wout": None,  # filled by caller
        }
        for nm, off in (("wq", 0), ("wk", N_EMBED), ("wv", 2 * N_EMBED)):
            w = (WSCALE * W_qkv[:, off:][:, csl]).astype(np.float32)
            w_hi = w.astype(e4)
            w_lo = (w - w_hi.astype(np.float32)).astype(e4)
            m[nm] = np.ascontiguousarray(np.stack([w_hi, w_lo], axis=1))
        in_maps.append(m)
    return in_maps


def kernel(x, W_qkv, b_qkv, W_out, b_out):
    x = np.asarray(x, dtype=np.float32)
    W_qkv = np.asarray(W_qkv, dtype=np.float32)
    b_qkv = np.asarray(b_qkv, dtype=np.float32)
    W_out = np.asarray(W_out, dtype=np.float32)
    b_out = np.asarray(b_out, dtype=np.float32)

    nc = _get_program(S)
    in_maps = make_in_maps(x, W_qkv, b_qkv, S)
    bf16 = ml_dtypes.bfloat16
    for c in range(N_CORES):
        csl = slice(DCORE * c, DCORE * (c + 1))
        in_maps[c]["wout"] = np.ascontiguousarray(W_out[csl, :]).astype(bf16)

    res = run_bass_kernel_spmd(nc, in_maps, core_ids=list(range(N_CORES)))
    y = np.zeros((B * S, N_EMBED), dtype=np.float32)
    for r in res.results:
        y += r["y"].astype(np.float32)
    y += b_out[None, :] + b_qkv[2 * N_EMBED:] @ W_out
    return y.reshape(B, S, N_EMBED)
